# revision 1
# baseline (speedup 1.0000x reference)
"""CIN (Compressed Interaction Network) kernel for Trainium2, 8-core data parallel.

Reference computation (per batch element b, position d):
  hidden = x                                  # (39 fields)
  layer i: z[(m,n)] = x[m] * hidden[n]        # outer product over fields
           cur[o]   = relu(sum_c z[c] W_i[c,o] + b_i[o])   # 200 outs
           hidden, direct = cur[:100], cur[100:]  (layers 0,1);  direct = cur (layer 2)
  out[b, j] = sum_d concat(directs)[j, d]     # (2048, 400)

Strategy: batch sharded across 8 cores (256 batch each, rows = b*32+d -> 8192).
Channel-major layout everywhere: X (39p, rows), H (100p, rows), all bf16.
z built on VectorE as XR (X broadcast across partitions via DMA from DRAM)
times H (free-dim broadcast).  TensorE contracts z blocks against the
(statically reordered) weights with PSUM accumulation; ScalarE applies
bias+ReLU+cast; VectorE reduces over d into the output accumulator.
"""

import sys

sys.path.insert(0, '/opt/trn_rl_repo')

import numpy as np
import ml_dtypes

import concourse.bacc as bacc
import concourse.mybir as mybir
import concourse.tile as tile
from concourse import bass_utils

BF16 = ml_dtypes.bfloat16

NCORES = 8
B = 2048
BC = B // NCORES          # 256 batch per core
D = 32
ROWS = BC * D             # 8192
F0 = 39
FK = 100
O = 200
RT = 512                  # rows per tile
NRT = ROWS // RT          # 16
BPT = RT // D             # batches per row tile = 16
G0 = 3                    # layer-0: 3 m-groups per K-block
P0 = G0 * F0              # 117
KB0 = F0 // G0            # 13
BL = 125                  # layers-1/2: packed K-block size (4-phase H windows)
NBL = 32                  # ceil(3900 / BL)
PAD_CH = BL * NBL         # 4000 (rows 3900.. are zero-padded in W)
NPH = 4                   # H phase offsets {0, 25, 50, 75}
BPP = NBL // NPH          # blocks per phase = 8
PERM = [q + NPH * i for q in range(NPH) for i in range(BPP)]  # phase-major block order
KZ0 = 4                   # layer-0 z first-chunk blocks (early PE start)

_cached = {}


def _emit(tc, outs, ins):
    nc = tc.nc
    x_d = ins['x_t']
    x0_d = ins['x0']
    xp_d = ins['xrp']
    w0_d = ins['w0']
    w1_d = ins['w1']
    w2_d = ins['w2']
    b_d = ins['bias']
    sel_d = ins['sel']
    out_d = outs['out']

    bf = mybir.dt.bfloat16
    f32 = mybir.dt.float32
    mult = mybir.AluOpType.mult
    add = mybir.AluOpType.add
    relu = mybir.ActivationFunctionType.Relu
    X = mybir.AxisListType.X

    import contextlib
    ctx = contextlib.ExitStack()
    with ctx:
        const = ctx.enter_context(tc.tile_pool(name="const", bufs=1))
        accp = ctx.enter_context(tc.tile_pool(name="acc", bufs=1))
        xrp = ctx.enter_context(tc.tile_pool(name="xr", bufs=2))
        xr0p = ctx.enter_context(tc.tile_pool(name="xr0", bufs=1))
        xt0p = ctx.enter_context(tc.tile_pool(name="xt0", bufs=1))
        z0p = ctx.enter_context(tc.tile_pool(name="z0", bufs=1))
        zp = ctx.enter_context(tc.tile_pool(name="z", bufs=5))
        hp = ctx.enter_context(tc.tile_pool(name="h", bufs=3))
        dp = ctx.enter_context(tc.tile_pool(name="d", bufs=2))
        psum = ctx.enter_context(tc.tile_pool(name="ps", bufs=6, space="PSUM"))
        selps = ctx.enter_context(tc.tile_pool(name="selps", bufs=2, space="PSUM"))

        # resident weights / bias (DMAs emitted after the critical rt=0 loads)
        w0_sb = const.tile([P0, KB0, O], bf, tag="w0")
        w1_sb = const.tile([BL, NBL, O], bf, tag="w1")
        w2_sb = const.tile([BL, NBL, O], bf, tag="w2")
        b_sb = const.tile([FK, 6], f32, tag="bias")
        sel_sb = const.tile([FK, NPH, BL], bf, tag="sel")

        # output accumulators (j-group on partitions, batch on free)
        acc = [accp.tile([FK, BC], f32, tag=f"acc{i}", name=f"acc{i}") for i in range(4)]

        def load_l0(rt, after_first=None):
            xr0 = xr0p.tile([P0, KB0, RT], bf, tag="xr0", name=f"xr0_{rt}")
            for dmi in range(G0):
                nc.sync.dma_start(
                    xr0[dmi * F0:(dmi + 1) * F0, :KZ0, :],
                    x0_d[rt, dmi, :KZ0][None, :, :].to_broadcast((F0, KZ0, RT)))
            if after_first is not None:
                after_first()
            for dmi in range(G0):
                nc.sync.dma_start(
                    xr0[dmi * F0:(dmi + 1) * F0, KZ0:, :],
                    x0_d[rt, dmi, KZ0:][None, :, :].to_broadcast((F0, KB0 - KZ0, RT)))
            xt0 = xt0p.tile([P0, RT], bf, tag="xt0", name=f"xt0_{rt}")
            nc.scalar.dma_start(xt0, x_d[rt][None, :, :].to_broadcast((G0, F0, RT)))
            return xr0, xt0

        def build_z0(l0t, rt):
            xr0, xt0 = l0t
            z0 = z0p.tile([P0, KB0, RT], bf, tag="z0", name=f"z0_{rt}")
            nc.vector.tensor_tensor(
                z0[:, :KZ0, :], xr0[:, :KZ0, :],
                xt0[:, None, :].to_broadcast((P0, KZ0, RT)), mult)
            nc.vector.tensor_tensor(
                z0[:, KZ0:, :], xr0[:, KZ0:, :],
                xt0[:, None, :].to_broadcast((P0, KB0 - KZ0, RT)), mult)
            return z0

        def load_xr(rt):
            xr = xrp.tile([BL, NBL, RT], bf, tag="xr", name=f"xr_{rt}")
            for q in range(NPH):
                sl = slice(q * BPP, (q + 1) * BPP)
                nc.gpsimd.dma_start(xr[:, sl, :], xp_d[rt, :, sl, :])
            return xr

        def make_phases(h_sb, li, rt):
            """ph_q[p] = H[(25q + p) mod 100] via 0/1 selection matmuls (exact)."""
            phs = []
            for q in range(NPH):
                psq = selps.tile([BL, RT], f32, tag="selps", name=f"sps{q}_{li}_{rt}")
                nc.tensor.matmul(psq, sel_sb[:, q, :], h_sb, start=True, stop=True)
                t = hp.tile([BL, RT], bf, tag=f"ph{q}", name=f"ph{q}_{li}_{rt}")
                nc.scalar.copy(t, psq)
                phs.append(t)
            return phs

        def l0_section(z0, rt):
            """L0 matmuls + relu + d0-reduce for row-tile rt; returns h1."""
            bs = slice(rt * BPT, (rt + 1) * BPT)
            ps0 = [psum.tile([FK, RT], f32, tag="ps", name=f"ps0_{rt}_{t}")
                   for t in range(2)]
            for t in range(2):
                for kb in range(KB0):
                    nc.tensor.matmul(ps0[t], w0_sb[:, kb, t * FK:(t + 1) * FK],
                                     z0[:, kb, :], start=(kb == 0),
                                     stop=(kb == KB0 - 1))
            h1 = hp.tile([FK, RT], bf, tag="h", name=f"h1_{rt}")
            nc.scalar.activation(h1, ps0[0], relu, bias=b_sb[:, 0:1])
            h1phs = make_phases(h1, 0, rt)
            d0 = dp.tile([FK, RT], bf, tag="d", name=f"d0_{rt}")
            nc.scalar.activation(d0, ps0[1], relu, bias=b_sb[:, 1:2])

            def d0_reduce():
                nc.vector.tensor_reduce(
                    acc[0][:, bs], d0.rearrange("o (g f) -> o g f", f=D), X, add)
            return h1phs, d0_reduce

        def layer_section(li, rt, xr, hphs, mid_hook=None):
            """Packed z-blocks + matmuls for layer li+1 (1 or 2) of row-tile rt."""
            bs = slice(rt * BPT, (rt + 1) * BPT)
            w_sb = w1_sb if li == 0 else w2_sb
            bcol = 2 + 2 * li
            ps = [psum.tile([FK, RT], f32, tag="ps", name=f"psl{li}_{rt}_{t}")
                  for t in range(2)]
            for q in range(NPH):
                zt = zp.tile([BL, BPP, RT], bf, tag="z", name=f"z{li}_{rt}_{q}")
                hb = hphs[q]
                xq = xr[:, q * BPP:(q + 1) * BPP, :]
                if q == 0:
                    # small first chunk so the PE refills quickly after the barrier
                    nc.vector.tensor_tensor(
                        zt[:, :2, :], xq[:, :2, :],
                        hb[:, None, :].to_broadcast((BL, 2, RT)), mult)
                    nc.vector.tensor_tensor(
                        zt[:, 2:, :], xq[:, 2:, :],
                        hb[:, None, :].to_broadcast((BL, BPP - 2, RT)), mult)
                else:
                    nc.vector.tensor_tensor(
                        zt, xq, hb[:, None, :].to_broadcast((BL, BPP, RT)), mult)
                for t in range(2):
                    for i in range(BPP):
                        kq = q * BPP + i
                        nc.tensor.matmul(ps[t], w_sb[:, kq, t * FK:(t + 1) * FK],
                                         zt[:, i, :], start=(q == 0 and i == 0),
                                         stop=(q == NPH - 1 and i == BPP - 1))
                if q == 1 and mid_hook is not None:
                    mid_hook()
            if li == 0:
                h2 = hp.tile([FK, RT], bf, tag="h", name=f"h2_{rt}")
                nc.scalar.activation(h2, ps[0], relu, bias=b_sb[:, bcol:bcol + 1])
                d1 = dp.tile([FK, RT], bf, tag="d", name=f"d1_{rt}")
                nc.scalar.activation(d1, ps[1], relu, bias=b_sb[:, bcol + 1:bcol + 2])
                nc.vector.tensor_reduce(
                    acc[1][:, bs], d1.rearrange("o (g f) -> o g f", f=D), X, add)
                return lambda: make_phases(h2, 1, rt)
            for t in range(2):
                d2 = dp.tile([FK, RT], bf, tag="d", name=f"d2_{rt}_{t}")
                nc.scalar.activation(d2, ps[t], relu,
                                     bias=b_sb[:, bcol + t:bcol + t + 1])
                nc.vector.tensor_reduce(
                    acc[2 + t][:, bs], d2.rearrange("o (g f) -> o g f", f=D),
                    X, add)
            return None

        # ---- software pipeline: [L1(rt) | L0(rt+1) | L2(rt)] per iteration
        l0t = load_l0(0, after_first=lambda: nc.sync.dma_start(w0_sb, w0_d))
        nc.sync.dma_start(b_sb, b_d)
        nc.sync.dma_start(sel_sb, sel_d)
        nc.scalar.dma_start(w1_sb, w1_d)
        nc.scalar.dma_start(w2_sb, w2_d)
        xr_cur = load_xr(0)
        z0_cur = build_z0(l0t, 0)
        h1, d0red = l0_section(z0_cur, 0)
        d0red()
        l0t_next = load_l0(1)
        state = {'z0_next': None}
        d0red_next = None

        for rt in range(NRT):
            xr_next = load_xr(rt + 1) if rt + 1 < NRT else None

            def mid_hook(rt=rt):
                if rt + 1 < NRT:
                    state['z0_next'] = build_z0(l0t_next, rt + 1)

            h2phs_fn = layer_section(0, rt, xr_cur, h1, mid_hook=mid_hook)
            h2phs = h2phs_fn()
            if rt + 1 < NRT:
                h1, d0red_next = l0_section(state['z0_next'], rt + 1)
                if rt + 2 < NRT:
                    l0t_next = load_l0(rt + 2)
            else:
                d0red_next = None
            layer_section(1, rt, xr_cur, h2phs, mid_hook=d0red_next)
            xr_cur = xr_next

        for i in range(4):
            nc.sync.dma_start(out_d[i * FK:(i + 1) * FK, :], acc[i])


def _pack_w(W):
    Wp = np.zeros((PAD_CH, O), np.float32)
    Wp[:F0 * FK] = W
    return np.ascontiguousarray(
        Wp.reshape(NBL, BL, O)[PERM].transpose(1, 0, 2)).astype(BF16)


def _prep_weights(W0, W1, W2, b0, b1, b2):
    w0 = np.ascontiguousarray(
        W0.reshape(KB0, P0, O).transpose(1, 0, 2)).astype(BF16)
    w1 = _pack_w(W1)
    w2 = _pack_w(W2)
    bias = np.ascontiguousarray(
        np.stack([b0, b1, b2]).reshape(3, 2, FK).transpose(2, 0, 1).reshape(FK, 6)
    ).astype(np.float32)
    sel = np.zeros((FK, NPH, BL), np.float32)
    q_, p_ = np.meshgrid(np.arange(NPH), np.arange(BL), indexing='ij')
    sel[(25 * q_ + p_) % FK, q_, p_] = 1.0
    return w0, w1, w2, bias, sel.astype(BF16)


def _prep_x_shard(x, c):
    xs = x[c * BC:(c + 1) * BC]                      # (BC, 39, 32)
    xt = xs.transpose(1, 0, 2).reshape(F0, ROWS)          # (39, 8192)
    x_t = np.ascontiguousarray(
        xt.reshape(F0, NRT, RT).transpose(1, 0, 2)).astype(BF16)
    x_perm = np.ascontiguousarray(
        xt.reshape(KB0, G0, NRT, RT).transpose(2, 1, 0, 3)).astype(BF16)
    mmap = np.minimum(np.arange(PAD_CH) // FK, F0 - 1).reshape(NBL, BL)[PERM]
    xrp = np.ascontiguousarray(
        x_t[:, mmap, :].transpose(0, 2, 1, 3))     # (NRT, BL, NBL, RT) phase-major
    return {'x_t': x_t, 'x0': x_perm, 'xrp': xrp}


def _build():
    if 'nc' in _cached:
        return _cached['nc']
    nc = bacc.Bacc("TRN2", target_bir_lowering=False, debug=False,
                   enable_asserts=False, num_devices=NCORES)
    ins = {
        'x_t': nc.dram_tensor("x_t", (NRT, F0, RT), mybir.dt.bfloat16,
                              kind="ExternalInput").ap(),
        'x0': nc.dram_tensor("x0", (NRT, G0, KB0, RT), mybir.dt.bfloat16,
                             kind="ExternalInput").ap(),
        'xrp': nc.dram_tensor("xrp", (NRT, BL, NBL, RT), mybir.dt.bfloat16,
                              kind="ExternalInput").ap(),
        'w0': nc.dram_tensor("w0", (P0, KB0, O), mybir.dt.bfloat16,
                             kind="ExternalInput").ap(),
        'w1': nc.dram_tensor("w1", (BL, NBL, O), mybir.dt.bfloat16,
                             kind="ExternalInput").ap(),
        'w2': nc.dram_tensor("w2", (BL, NBL, O), mybir.dt.bfloat16,
                             kind="ExternalInput").ap(),
        'bias': nc.dram_tensor("bias", (FK, 6), mybir.dt.float32,
                               kind="ExternalInput").ap(),
        'sel': nc.dram_tensor("sel", (FK, NPH, BL), mybir.dt.bfloat16,
                              kind="ExternalInput").ap(),
    }
    outs = {
        'out': nc.dram_tensor("out", (4 * FK, BC), mybir.dt.float32,
                              kind="ExternalOutput").ap(),
    }
    with tile.TileContext(nc, trace_sim=False) as tc:
        _emit(tc, outs, ins)
    nc.compile()
    _cached['nc'] = nc
    return nc


def kernel(x, W0, W1, W2, b0, b1, b2):
    nc = _build()
    w0, w1, w2, bias, sel = _prep_weights(
        np.asarray(W0, np.float32), np.asarray(W1, np.float32),
        np.asarray(W2, np.float32), np.asarray(b0, np.float32),
        np.asarray(b1, np.float32), np.asarray(b2, np.float32))
    x = np.asarray(x, np.float32)
    in_maps = []
    for c in range(NCORES):
        in_maps.append({
            **_prep_x_shard(x, c),
            'w0': w0, 'w1': w1, 'w2': w2, 'bias': bias, 'sel': sel,
        })
    res = bass_utils.run_bass_kernel_spmd(
        nc, in_maps, core_ids=list(range(NCORES)))
    out = np.empty((B, 4 * FK), np.float32)
    for c in range(NCORES):
        out[c * BC:(c + 1) * BC, :] = res.results[c]['out'].T
    return out



# revision 2
# speedup vs baseline: 1.0252x; 1.0252x over previous
"""CIN (Compressed Interaction Network) kernel for Trainium2, 8-core data parallel.

v2: z-stationary matmul formulation.

Reference computation (per batch element b, position d):
  hidden = x                                  # (39 fields)
  layer i: z[(m,n)] = x[m] * hidden[n]        # outer product over fields
           cur[o]   = relu(sum_c z[c] W_i[c,o] + b_i[o])   # 200 outs
           hidden, direct = cur[:100], cur[100:]  (layers 0,1);  direct = cur (layer 2)
  out[b, j] = sum_d concat(directs)[j, d]     # (2048, 400)

Strategy: batch sharded across 8 cores (256 batch each, rows = b*32+d -> 8192).
Matmuls run z-STATIONARY: lhsT = z block [K<=125 chans, M=128 rows] (M=128
exactly -> fast-weight-load), rhs = W [K, 200 outs] streams N=200 cycles and
covers BOTH output halves in one pass.  Outputs land [rows, 200] in PSUM;
bias is added by one extra K=1 matmul (ones x bias-row); ScalarE applies
ReLU -> SBUF bf16.  The h-half is transposed back to channel-major by PE
transpose, phase-replicated via 0/1 selection matmuls, and multiplied with
the broadcast-gathered X on VectorE to form the next layer's z.  The direct
halves are reduced over d by tiny [128,16] selection matmuls accumulating
into a per-row-tile [16, 400] PSUM accumulator DMA'd straight out.

X replication (xr[p, kq] = x[m(p,kq)]) exploits the phase-major block
permutation: within phase q the source row advances by exactly 5 per block,
so 9 broadcast DMAs per row tile (reading tiny 8-row slabs) build the 4MB
SBUF operand instead of streaming a 65MB host-precomputed gather from HBM.
Layer-0's z = x (x) x is precomputed on the host (it has no on-device
dependency) and DMA'd.
"""

import sys

sys.path.insert(0, '/opt/trn_rl_repo')

import numpy as np
import ml_dtypes

import concourse.bacc as bacc
import concourse.mybir as mybir
import concourse.tile as tile
from concourse import bass_utils

BF16 = ml_dtypes.bfloat16

NCORES = 8
B = 2048
BC = B // NCORES          # 256 batch per core
D = 32
ROWS = BC * D             # 8192
F0 = 39
FK = 100
O = 200
RT = 512                  # rows per tile
NRT = ROWS // RT          # 16
BPT = RT // D             # batches per row tile = 16
RB = 128                  # rows per matmul block (stationary M)
NRB = RT // RB            # 4
P0 = 128                  # layer-0 z partitions
KB0 = 7                   # layer-0 K blocks: symmetric z0 -> 780 channels (m<=n),
                          # folded W0'[(m,n)] = W0[m,n] + W0[n,m]; padded to 896
BL = 125                  # layers-1/2: packed K-block size (4-phase H windows)
NBL = 32                  # blocks
NPH = 4                   # H phase offsets {0, 25, 50, 75}
BPP = NBL // NPH          # blocks per phase = 8
PERM = [q + NPH * i for q in range(NPH) for i in range(BPP)]  # phase-major block order

# broadcast-gather pieces for xr[p, q*8+i, :] = x[min((125q+p)//100 + 5i, 38), :]
# (q, p0, p1, i0, i1, m_start);  m_start = row for i0 (rows advance by 5)
GATHERS = [
    (0, 0, 100, 0, 8, 0),
    (0, 100, 125, 0, 8, 1),
    (1, 0, 75, 0, 8, 1),
    (1, 75, 125, 0, 8, 2),
    (2, 0, 50, 0, 8, 2),
    (2, 50, 125, 0, 8, 3),
    (3, 0, 25, 0, 8, 3),
    (3, 25, 125, 0, 7, 4),
    (3, 25, 125, 7, 8, 38),   # clamped tail: every p reads row 38
]

_cached = {}


def _emit(tc, outs, ins, with_bias=True):
    nc = tc.nc
    z0_d = ins['z0']
    xsrc_d = ins['xsrc']
    w0_d = ins['w0']
    w1_d = ins['w1']
    w2_d = ins['w2']
    blob_d = ins['blob']
    out_d = outs['out']

    bf = mybir.dt.bfloat16
    f32 = mybir.dt.float32
    mult = mybir.AluOpType.mult
    relu = mybir.ActivationFunctionType.Relu

    import contextlib
    ctx = contextlib.ExitStack()
    with ctx:
        const = ctx.enter_context(tc.tile_pool(name="const", bufs=1))
        z0p = ctx.enter_context(tc.tile_pool(name="z0", bufs=2))
        xrp = ctx.enter_context(tc.tile_pool(name="xr", bufs=2))
        zp = ctx.enter_context(tc.tile_pool(name="z", bufs=4))
        php = ctx.enter_context(tc.tile_pool(name="ph", bufs=1))
        hcmp = ctx.enter_context(tc.tile_pool(name="hcm", bufs=1))
        relup = ctx.enter_context(tc.tile_pool(name="relu", bufs=8))
        outsb = ctx.enter_context(tc.tile_pool(name="outsb", bufs=2))
        mainps = ctx.enter_context(tc.tile_pool(name="mps", bufs=3, space="PSUM"))
        tps = ctx.enter_context(tc.tile_pool(name="tps", bufs=1, space="PSUM"))
        selps = ctx.enter_context(tc.tile_pool(name="selps", bufs=2, space="PSUM"))
        accp = ctx.enter_context(tc.tile_pool(name="acc", bufs=2, space="PSUM"))

        # resident weights / consts (small ones packed in one blob: cols
        # ident 0:128 | srb 128:192 | sel 192:704 | ones 704:832 | bias 832:1432)
        w0_sb = const.tile([P0, KB0, O], bf, tag="w0")
        w1_sb = const.tile([BL, NBL, O], bf, tag="w1")
        w2_sb = const.tile([BL, NBL, O], bf, tag="w2")
        blob_sb = const.tile([128, 1440], bf, tag="blob")
        id_sb = blob_sb[:, 0:128]
        srb_sb = blob_sb[:, 128:192].rearrange("p (a b) -> p a b", b=BPT)
        sel_sb = blob_sb[0:FK, 192:704].rearrange("p (q c) -> p q c", c=128)
        ones_sb = blob_sb[0:1, 704:832]
        bias_sb = blob_sb[0:1, 832:1432].rearrange("p (l c) -> p l c", c=O)

        def load_z0(rt):
            z0 = z0p.tile([P0, KB0, RT], bf, tag="z0", name=f"z0_{rt}")
            nc.sync.dma_start(z0, z0_d[rt])
            return z0

        def load_xr(rt, js=None, xr=None, engs=None):
            """Issue gather DMAs js (default all) for row tile rt into xr."""
            if xr is None:
                xr = xrp.tile([BL, NBL, RT], bf, tag="xr", name=f"xr_{rt}")
            for j in (range(len(GATHERS)) if js is None else js):
                q, p0, p1, i0, i1, _m = GATHERS[j]
                if engs is None:
                    eng = nc.gpsimd if j < 5 else nc.sync
                else:
                    eng = engs[j % len(engs)]
                eng.dma_start(
                    xr[p0:p1, q * BPP + i0:q * BPP + i1, :],
                    xsrc_d[rt, j, :i1 - i0, :][None, :, :].to_broadcast(
                        (p1 - p0, i1 - i0, RT)))
            return xr

        def instance(li, rt, zts, accps, xr, hooks=None):
            """One layer instance: z-stationary matmuls + post-processing.

            Small matmuls (transpose/sel/d-sum of rowblock rb) are emitted one
            rowblock behind the mains so they never stall on a just-issued
            ReLU.  sel/z-build run at half-tile width (256 rows): half h of
            the next layer's z feeds exactly rowblocks 2h and 2h+1 there, so
            z is ready well before it is consumed.
            """
            w_sb = (w0_sb, w1_sb, w2_sb)[li]
            nkb = KB0 if li == 0 else NBL
            if li < 2:
                ph = php.tile([BL, NPH, RT], bf, tag=f"ph{li}", name=f"ph{li}_{rt}")
                hcm = hcmp.tile([FK, RT], bf, tag=f"hcm{li}", name=f"hcm{li}_{rt}")
                tp = tps.tile([128, NRB, 128], bf, tag="tp", name=f"tp{li}_{rt}")
                znx = [zp.tile([BL, BPP, RT], bf, tag=f"z{li + 1}",
                               name=f"z{li + 1}_{rt}_{q}") for q in range(NPH)]
            else:
                ph = hcm = tp = znx = None
            rls = [None] * NRB
            dsl = slice(FK, O) if li < 2 else slice(0, O)
            dreg = slice(li * FK, (li + 1) * FK) if li < 2 else slice(O, 2 * O)

            def post_h(rb):
                rbsl = slice(rb * RB, (rb + 1) * RB)
                nc.tensor.transpose(tp[:, rb, :], rls[rb][:, 0:128], id_sb)
                nc.scalar.copy(hcm[:, rbsl], tp[0:FK, rb, :])

            def dsum(rb):
                nc.tensor.matmul(accps[:, dreg], srb_sb[:, rb, :], rls[rb][:, dsl],
                                 start=(rb == 0), stop=(rb == NRB - 1))

            def sel_half(h):
                hsl = slice(h * (RT // 2), (h + 1) * (RT // 2))
                for q in range(NPH):
                    sps = selps.tile([128, RT // 2], f32, tag="selps",
                                     name=f"sps{li}_{rt}_{h}_{q}")
                    nc.tensor.matmul(sps, sel_sb[:, q, :], hcm[:, hsl],
                                     start=True, stop=True)
                    nc.scalar.copy(ph[:, q, hsl], sps[0:BL, :])
                    nc.vector.tensor_tensor(
                        znx[q][:, :, hsl], xr[:, q * BPP:(q + 1) * BPP, hsl],
                        ph[:, q, hsl][:, None, :].to_broadcast(
                            (BL, BPP, RT // 2)), mult)

            for rb in range(NRB):
                rbsl = slice(rb * RB, (rb + 1) * RB)
                ps = mainps.tile([RB, O], f32, tag="mps", name=f"ps{li}_{rt}_{rb}")
                for kb in range(nkb):
                    if li == 0:
                        lhs = zts[:, kb, rbsl]
                    else:
                        lhs = zts[kb // BPP][:, kb % BPP, rbsl]
                    nc.tensor.matmul(ps, lhs, w_sb[:, kb, :],
                                     start=(kb == 0),
                                     stop=(kb == nkb - 1 and not with_bias))
                if with_bias:
                    nc.tensor.matmul(ps, ones_sb, bias_sb[:, li, :],
                                     start=False, stop=True)
                rl = relup.tile([RB, O], bf, tag="relu", name=f"rl{li}_{rt}_{rb}")
                nc.scalar.activation(rl, ps, relu)
                rls[rb] = rl
                if rb >= 1:
                    if li < 2:
                        post_h(rb - 1)
                    dsum(rb - 1)
                    if li < 2 and rb == 2:
                        sel_half(0)
                if hooks and rb in hooks:
                    hooks[rb]()
            if li < 2:
                post_h(NRB - 1)
                sel_half(1)
            dsum(NRB - 1)
            return znx

        # ---- prologue.  DMAs split 3 ways; PE warms the HAM clock gate with
        # dummy matmuls on the identity tile while the first inputs stream in.
        nc.scalar.dma_start(blob_sb, blob_d)
        nc.sync.dma_start(w0_sb, w0_d)
        z0t = load_z0(0)
        xr_cur = load_xr(0, engs=[nc.gpsimd, nc.sync, nc.scalar])
        nc.gpsimd.dma_start(w1_sb, w1_d)
        nc.scalar.dma_start(w2_sb, w2_d)
        wt = mainps.tile([RB, O], f32, tag="mps", name="warm")
        for i in range(96):
            nc.tensor.matmul(wt[:, 0:64], id_sb, id_sb[:, 0:64],
                             start=True, stop=True)
        acc_cur = accp.tile([BPT, 2 * O], f32, tag="acc", name="acc_0")
        z1t = instance(0, 0, z0t, acc_cur, xr_cur)
        z0t_next = load_z0(1)
        xr_next = load_xr(1)

        for rt in range(NRT):
            z2t = instance(1, rt, z1t, acc_cur, xr_cur)
            if rt + 1 < NRT:
                acc_next = accp.tile([BPT, 2 * O], f32, tag="acc",
                                     name=f"acc_{rt + 1}")
                z1t = instance(0, rt + 1, z0t_next, acc_next, xr_next)
            # prefetch rt+2 from inside the L2 instance, after the consumers
            # of the buffers being overwritten have drained (waits ~0 there,
            # and gpsimd/sync queues carry nothing latency-critical).
            hooks = {}
            if rt + 2 < NRT:
                xr_nn = xrp.tile([BL, NBL, RT], bf, tag="xr",
                                 name=f"xr_{rt + 2}")
                hooks[1] = lambda: load_xr(rt + 2, js=range(0, 5), xr=xr_nn)
                hooks[3] = lambda: load_xr(rt + 2, js=range(5, 9), xr=xr_nn)
            else:
                xr_nn = None
            instance(2, rt, z2t, acc_cur, xr_cur, hooks=hooks)
            ost = outsb.tile([BPT, 2 * O], f32, tag="ost", name=f"ost_{rt}")
            nc.scalar.copy(ost, acc_cur)
            nc.sync.dma_start(out_d[rt], ost)
            if rt + 2 < NRT:
                z0t_next = load_z0(rt + 2)
            if rt + 1 < NRT:
                acc_cur = acc_next
                xr_cur = xr_next
                xr_next = xr_nn


def _pack_w(W):
    Wp = np.zeros((BL * NBL, O), np.float32)
    Wp[:F0 * FK] = W
    return np.ascontiguousarray(
        Wp.reshape(NBL, BL, O)[PERM].transpose(1, 0, 2)).astype(BF16)


def _prep_weights(W0, W1, W2, b0, b1, b2):
    mi, ni = np.triu_indices(F0)
    W0f = W0.reshape(F0, F0, O)
    W0sym = W0f[mi, ni] + np.where((mi != ni)[:, None], W0f[ni, mi], 0.0)
    W0p = np.zeros((P0 * KB0, O), np.float32)
    W0p[:len(mi)] = W0sym
    w0 = np.ascontiguousarray(
        W0p.reshape(KB0, P0, O).transpose(1, 0, 2)).astype(BF16)
    w1 = _pack_w(W1)
    w2 = _pack_w(W2)
    sel = np.zeros((FK, NPH, 128), np.float32)
    q_, p_ = np.meshgrid(np.arange(NPH), np.arange(BL), indexing='ij')
    sel[(25 * q_ + p_) % FK, q_, p_] = 1.0
    srb = np.zeros((128, NRB, BPT), np.float32)
    r_ = np.arange(128)
    for rb in range(NRB):
        srb[r_, rb, rb * 4 + r_ // D] = 1.0
    blob = np.zeros((128, 1440), np.float32)
    blob[:, 0:128] = np.eye(128)
    blob[:, 128:192] = srb.reshape(128, -1)
    blob[:FK, 192:704] = sel.reshape(FK, -1)
    blob[0, 704:832] = 1.0
    blob[0, 832:1432] = np.stack([b0, b1, b2]).reshape(-1)
    return w0, w1, w2, blob.astype(BF16)


def _prep_x_shard(x, c):
    xs = x[c * BC:(c + 1) * BC]                           # (BC, 39, 32)
    xt = xs.transpose(1, 0, 2).reshape(F0, ROWS)          # (39, 8192)
    xtr = xt.reshape(F0, NRT, RT)
    # z0: symmetric outer product, upper-triangle channels (m <= n) only;
    # channel c -> (kb, p) with c = 128*kb + p
    mi, ni = np.triu_indices(F0)
    z0 = np.zeros((KB0 * P0, ROWS), np.float32)
    z0[:len(mi)] = xt[mi] * xt[ni]
    z0 = z0.reshape(KB0, P0, NRT, RT)
    z0 = np.ascontiguousarray(z0.transpose(2, 1, 0, 3)).astype(BF16)
    # gather source slabs: xsrc[rt, j, i-i0, :] = x[min(m_start + 5*(i-i0)... , 38)]
    xsrc = np.zeros((NRT, len(GATHERS), 8, RT), np.float32)
    for j, (q, p0, p1, i0, i1, m0) in enumerate(GATHERS):
        for k in range(i1 - i0):
            xsrc[:, j, k, :] = xtr[min(m0 + 5 * k, F0 - 1)]
    return {'z0': z0, 'xsrc': xsrc.astype(BF16)}


def _build(with_bias=True):
    key = f'nc_{with_bias}'
    if key in _cached:
        return _cached[key]
    nc = bacc.Bacc("TRN2", target_bir_lowering=False, debug=False,
                   enable_asserts=False, num_devices=NCORES)
    ins = {
        'z0': nc.dram_tensor("z0", (NRT, P0, KB0, RT), mybir.dt.bfloat16,
                             kind="ExternalInput").ap(),
        'xsrc': nc.dram_tensor("xsrc", (NRT, len(GATHERS), 8, RT),
                               mybir.dt.bfloat16, kind="ExternalInput").ap(),
        'w0': nc.dram_tensor("w0", (P0, KB0, O), mybir.dt.bfloat16,
                             kind="ExternalInput").ap(),
        'w1': nc.dram_tensor("w1", (BL, NBL, O), mybir.dt.bfloat16,
                             kind="ExternalInput").ap(),
        'w2': nc.dram_tensor("w2", (BL, NBL, O), mybir.dt.bfloat16,
                             kind="ExternalInput").ap(),
        'blob': nc.dram_tensor("blob", (128, 1440), mybir.dt.bfloat16,
                               kind="ExternalInput").ap(),
    }
    outs = {
        'out': nc.dram_tensor("out", (NRT, BPT, 2 * O), mybir.dt.float32,
                              kind="ExternalOutput").ap(),
    }
    with tile.TileContext(nc, trace_sim=False) as tc:
        _emit(tc, outs, ins, with_bias=with_bias)
    nc.compile()
    _cached[key] = nc
    return nc


def kernel(x, W0, W1, W2, b0, b1, b2):
    bias_zero = not (np.any(b0) or np.any(b1) or np.any(b2))
    nc = _build(with_bias=not bias_zero)
    w0, w1, w2, blob = _prep_weights(
        np.asarray(W0, np.float32), np.asarray(W1, np.float32),
        np.asarray(W2, np.float32), np.asarray(b0, np.float32),
        np.asarray(b1, np.float32), np.asarray(b2, np.float32))
    x = np.asarray(x, np.float32)
    in_maps = []
    for c in range(NCORES):
        in_maps.append({
            **_prep_x_shard(x, c),
            'w0': w0, 'w1': w1, 'w2': w2, 'blob': blob,
        })
    res = bass_utils.run_bass_kernel_spmd(
        nc, in_maps, core_ids=list(range(NCORES)))
    out = np.empty((B, 2 * O), np.float32)
    for c in range(NCORES):
        out[c * BC:(c + 1) * BC, :] = res.results[c]['out'].reshape(BC, 2 * O)
    return out


# revision 3
# speedup vs baseline: 1.0385x; 1.0129x over previous
"""CIN (Compressed Interaction Network) kernel for Trainium2, 8-core data parallel.

v2: z-stationary matmul formulation.

Reference computation (per batch element b, position d):
  hidden = x                                  # (39 fields)
  layer i: z[(m,n)] = x[m] * hidden[n]        # outer product over fields
           cur[o]   = relu(sum_c z[c] W_i[c,o] + b_i[o])   # 200 outs
           hidden, direct = cur[:100], cur[100:]  (layers 0,1);  direct = cur (layer 2)
  out[b, j] = sum_d concat(directs)[j, d]     # (2048, 400)

Strategy: batch sharded across 8 cores (256 batch each, rows = b*32+d -> 8192).
Matmuls run z-STATIONARY: lhsT = z block [K<=125 chans, M=128 rows] (M=128
exactly -> fast-weight-load), rhs = W [K, 200 outs] streams N=200 cycles and
covers BOTH output halves in one pass.  Outputs land [rows, 200] in PSUM;
bias is added by one extra K=1 matmul (ones x bias-row); ScalarE applies
ReLU -> SBUF bf16.  The h-half is transposed back to channel-major by PE
transpose, phase-replicated via 0/1 selection matmuls, and multiplied with
the broadcast-gathered X on VectorE to form the next layer's z.  The direct
halves are reduced over d by tiny [128,16] selection matmuls accumulating
into a per-row-tile [16, 400] PSUM accumulator DMA'd straight out.

X replication (xr[p, kq] = x[m(p,kq)]) exploits the phase-major block
permutation: within phase q the source row advances by exactly 5 per block,
so 9 broadcast DMAs per row tile (reading tiny 8-row slabs) build the 4MB
SBUF operand instead of streaming a 65MB host-precomputed gather from HBM.
Layer-0's z = x (x) x is precomputed on the host (it has no on-device
dependency) and DMA'd.
"""

import sys

sys.path.insert(0, '/opt/trn_rl_repo')

import numpy as np
import ml_dtypes

import concourse.bacc as bacc
import concourse.mybir as mybir
import concourse.tile as tile
from concourse import bass_utils

BF16 = ml_dtypes.bfloat16

NCORES = 8
B = 2048
BC = B // NCORES          # 256 batch per core
D = 32
ROWS = BC * D             # 8192
F0 = 39
FK = 100
O = 200
RT = 512                  # rows per tile
NRT = ROWS // RT          # 16
BPT = RT // D             # batches per row tile = 16
RB = 128                  # rows per matmul block (stationary M)
NRB = RT // RB            # 4
P0 = 128                  # layer-0 z partitions
KB0 = 7                   # layer-0 K blocks: symmetric z0 -> 780 channels (m<=n),
                          # folded W0'[(m,n)] = W0[m,n] + W0[n,m]; padded to 896
BL = 125                  # layers-1/2: packed K-block size (4-phase H windows)
NBL = 32                  # blocks
NPH = 4                   # H phase offsets {0, 25, 50, 75}
BPP = NBL // NPH          # blocks per phase = 8
PERM = [q + NPH * i for q in range(NPH) for i in range(BPP)]  # phase-major block order

# broadcast-gather pieces for xr[p, q*8+i, :] = x[min((125q+p)//100 + 5i, 38), :]
# (q, p0, p1, i0, i1, m_start);  m_start = row for i0 (rows advance by 5)
GATHERS = [
    (0, 0, 100, 0, 8, 0),
    (0, 100, 125, 0, 8, 1),
    (1, 0, 75, 0, 8, 1),
    (1, 75, 125, 0, 8, 2),
    (2, 0, 50, 0, 8, 2),
    (2, 50, 125, 0, 8, 3),
    (3, 0, 25, 0, 8, 3),
    (3, 25, 125, 0, 7, 4),
    (3, 25, 125, 7, 8, 38),   # clamped tail: every p reads row 38
]

_cached = {}


def _emit(tc, outs, ins, with_bias=True):
    nc = tc.nc
    z0_d = ins['z0']
    xsrc_d = ins['xsrc']
    w0_d = ins['w0']
    w1_d = ins['w1']
    w2_d = ins['w2']
    blob_d = ins['blob']
    out_d = outs['out']

    bf = mybir.dt.bfloat16
    f32 = mybir.dt.float32
    mult = mybir.AluOpType.mult
    relu = mybir.ActivationFunctionType.Relu

    import contextlib
    ctx = contextlib.ExitStack()
    with ctx:
        const = ctx.enter_context(tc.tile_pool(name="const", bufs=1))
        z0p = ctx.enter_context(tc.tile_pool(name="z0", bufs=2))
        xrp = ctx.enter_context(tc.tile_pool(name="xr", bufs=2))
        zp = ctx.enter_context(tc.tile_pool(name="z", bufs=4))
        php = ctx.enter_context(tc.tile_pool(name="ph", bufs=1))
        hcmp = ctx.enter_context(tc.tile_pool(name="hcm", bufs=1))
        relup = ctx.enter_context(tc.tile_pool(name="relu", bufs=8))
        outsb = ctx.enter_context(tc.tile_pool(name="outsb", bufs=2))
        mainps = ctx.enter_context(tc.tile_pool(name="mps", bufs=3, space="PSUM"))
        tps = ctx.enter_context(tc.tile_pool(name="tps", bufs=1, space="PSUM"))
        selps = ctx.enter_context(tc.tile_pool(name="selps", bufs=2, space="PSUM"))
        accp = ctx.enter_context(tc.tile_pool(name="acc", bufs=2, space="PSUM"))

        # resident weights / consts (small ones packed in one blob: cols
        # ident 0:128 | srb 128:192 | sel 192:704 | ones 704:832 | bias 832:1432)
        w0_sb = const.tile([P0, KB0, O], bf, tag="w0")
        w1_sb = const.tile([BL, NBL, O], bf, tag="w1")
        w2_sb = const.tile([BL, NBL, O], bf, tag="w2")
        blob_sb = const.tile([128, 1440], bf, tag="blob")
        id_sb = blob_sb[:, 0:128]
        srb_sb = blob_sb[:, 128:192].rearrange("p (a b) -> p a b", b=BPT)
        sel_sb = blob_sb[0:FK, 192:704].rearrange("p (q c) -> p q c", c=128)
        ones_sb = blob_sb[0:1, 704:832]
        bias_sb = blob_sb[0:1, 832:1432].rearrange("p (l c) -> p l c", c=O)

        def load_z0(rt):
            z0 = z0p.tile([P0, KB0, RT], bf, tag="z0", name=f"z0_{rt}")
            nc.sync.dma_start(z0[:, :4, :], z0_d[rt, :, :4, :])
            nc.sync.dma_start(z0[:, 4:, :], z0_d[rt, :, 4:, :])
            return z0

        def load_xr(rt, js=None, xr=None, engs=None):
            """Issue gather DMAs js (default all) for row tile rt into xr."""
            if xr is None:
                xr = xrp.tile([BL, NBL, RT], bf, tag="xr", name=f"xr_{rt}")
            for j in (range(len(GATHERS)) if js is None else js):
                q, p0, p1, i0, i1, _m = GATHERS[j]
                if engs is None:
                    eng = nc.gpsimd if j < 5 else nc.sync
                else:
                    eng = engs[j % len(engs)]
                eng.dma_start(
                    xr[p0:p1, q * BPP + i0:q * BPP + i1, :],
                    xsrc_d[rt, j, :i1 - i0, :][None, :, :].to_broadcast(
                        (p1 - p0, i1 - i0, RT)))
            return xr

        def instance(li, rt, zts, accps, xr, hooks=None):
            """One layer instance: z-stationary matmuls + post-processing.

            Small matmuls (transpose/sel/d-sum of rowblock rb) are emitted one
            rowblock behind the mains so they never stall on a just-issued
            ReLU.  sel/z-build run at half-tile width (256 rows): half h of
            the next layer's z feeds exactly rowblocks 2h and 2h+1 there, so
            z is ready well before it is consumed.
            """
            w_sb = (w0_sb, w1_sb, w2_sb)[li]
            nkb = KB0 if li == 0 else NBL
            if li < 2:
                ph = php.tile([BL, NPH, RT], bf, tag=f"ph{li}", name=f"ph{li}_{rt}")
                hcm = hcmp.tile([FK, RT], bf, tag=f"hcm{li}", name=f"hcm{li}_{rt}")
                tp = tps.tile([128, NRB, 128], bf, tag="tp", name=f"tp{li}_{rt}")
                znx = [zp.tile([BL, BPP, RT], bf, tag=f"z{li + 1}",
                               name=f"z{li + 1}_{rt}_{q}") for q in range(NPH)]
            else:
                ph = hcm = tp = znx = None
            rls = [None] * NRB
            dsl = slice(FK, O) if li < 2 else slice(0, O)
            dreg = slice(li * FK, (li + 1) * FK) if li < 2 else slice(O, 2 * O)

            def post_h(rb):
                rbsl = slice(rb * RB, (rb + 1) * RB)
                nc.tensor.transpose(tp[:, rb, :], rls[rb][:, 0:128], id_sb)
                nc.scalar.copy(hcm[:, rbsl], tp[0:FK, rb, :])

            def dsum(rb):
                nc.tensor.matmul(accps[:, dreg], srb_sb[:, rb, :], rls[rb][:, dsl],
                                 start=(rb == 0), stop=(rb == NRB - 1))

            def sel_half(h):
                hsl = slice(h * (RT // 2), (h + 1) * (RT // 2))
                for q in range(NPH):
                    sps = selps.tile([128, RT // 2], f32, tag="selps",
                                     name=f"sps{li}_{rt}_{h}_{q}")
                    nc.tensor.matmul(sps, sel_sb[:, q, :], hcm[:, hsl],
                                     start=True, stop=True)
                    nc.scalar.copy(ph[:, q, hsl], sps[0:BL, :])
                    nc.vector.tensor_tensor(
                        znx[q][:, :, hsl], xr[:, q * BPP:(q + 1) * BPP, hsl],
                        ph[:, q, hsl][:, None, :].to_broadcast(
                            (BL, BPP, RT // 2)), mult)

            for rb in range(NRB):
                rbsl = slice(rb * RB, (rb + 1) * RB)
                ps = mainps.tile([RB, O], f32, tag="mps", name=f"ps{li}_{rt}_{rb}")
                for kb in range(nkb):
                    if li == 0:
                        lhs = zts[:, kb, rbsl]
                    else:
                        lhs = zts[kb // BPP][:, kb % BPP, rbsl]
                    nc.tensor.matmul(ps, lhs, w_sb[:, kb, :],
                                     start=(kb == 0),
                                     stop=(kb == nkb - 1 and not with_bias))
                if with_bias:
                    nc.tensor.matmul(ps, ones_sb, bias_sb[:, li, :],
                                     start=False, stop=True)
                rl = relup.tile([RB, O], bf, tag="relu", name=f"rl{li}_{rt}_{rb}")
                nc.scalar.activation(rl, ps, relu)
                rls[rb] = rl
                if rb >= 1:
                    if li < 2:
                        post_h(rb - 1)
                    dsum(rb - 1)
                    if li < 2 and rb == 2:
                        sel_half(0)
                if hooks and rb in hooks:
                    hooks[rb]()
            if li < 2:
                post_h(NRB - 1)
                sel_half(1)
            dsum(NRB - 1)
            return znx

        # ---- prologue.  DMAs split 3 ways; PE warms the HAM clock gate with
        # dummy matmuls on the identity tile while the first inputs stream in.
        nc.scalar.dma_start(blob_sb, blob_d)
        nc.sync.dma_start(w0_sb, w0_d)
        z0t = load_z0(0)
        xr_cur = load_xr(0, engs=[nc.gpsimd, nc.sync, nc.scalar])
        nc.gpsimd.dma_start(w1_sb, w1_d)
        nc.scalar.dma_start(w2_sb, w2_d)
        wt = mainps.tile([RB, O], f32, tag="mps", name="warm")
        for i in range(96):
            nc.tensor.matmul(wt[:, 0:64], id_sb, id_sb[:, 0:64],
                             start=True, stop=True)
        acc_cur = accp.tile([BPT, 2 * O], f32, tag="acc", name="acc_0")
        z1t = instance(0, 0, z0t, acc_cur, xr_cur)
        z0t_next = load_z0(1)
        xr_next = load_xr(1)

        for rt in range(NRT):
            z2t = instance(1, rt, z1t, acc_cur, xr_cur)
            if rt + 1 < NRT:
                acc_next = accp.tile([BPT, 2 * O], f32, tag="acc",
                                     name=f"acc_{rt + 1}")
                z1t = instance(0, rt + 1, z0t_next, acc_next, xr_next)
            # prefetch rt+2 from inside the L2 instance, after the consumers
            # of the buffers being overwritten have drained (waits ~0 there,
            # and gpsimd/sync queues carry nothing latency-critical).
            hooks = {}
            if rt + 2 < NRT:
                xr_nn = xrp.tile([BL, NBL, RT], bf, tag="xr",
                                 name=f"xr_{rt + 2}")
                hooks[1] = lambda: load_xr(rt + 2, js=range(0, 5), xr=xr_nn)
                hooks[3] = lambda: load_xr(rt + 2, js=range(5, 9), xr=xr_nn)
            else:
                xr_nn = None
            instance(2, rt, z2t, acc_cur, xr_cur, hooks=hooks)
            ost = outsb.tile([BPT, 2 * O], f32, tag="ost", name=f"ost_{rt}")
            nc.scalar.copy(ost, acc_cur)
            nc.sync.dma_start(out_d[rt], ost)
            if rt + 2 < NRT:
                z0t_next = load_z0(rt + 2)
            if rt + 1 < NRT:
                acc_cur = acc_next
                xr_cur = xr_next
                xr_next = xr_nn


def _pack_w(W):
    Wp = np.zeros((BL * NBL, O), np.float32)
    Wp[:F0 * FK] = W
    return np.ascontiguousarray(
        Wp.reshape(NBL, BL, O)[PERM].transpose(1, 0, 2)).astype(BF16)


def _prep_weights(W0, W1, W2, b0, b1, b2):
    mi, ni = np.triu_indices(F0)
    W0f = W0.reshape(F0, F0, O)
    W0sym = W0f[mi, ni] + np.where((mi != ni)[:, None], W0f[ni, mi], 0.0)
    W0p = np.zeros((P0 * KB0, O), np.float32)
    W0p[:len(mi)] = W0sym
    w0 = np.ascontiguousarray(
        W0p.reshape(KB0, P0, O).transpose(1, 0, 2)).astype(BF16)
    w1 = _pack_w(W1)
    w2 = _pack_w(W2)
    sel = np.zeros((FK, NPH, 128), np.float32)
    q_, p_ = np.meshgrid(np.arange(NPH), np.arange(BL), indexing='ij')
    sel[(25 * q_ + p_) % FK, q_, p_] = 1.0
    srb = np.zeros((128, NRB, BPT), np.float32)
    r_ = np.arange(128)
    for rb in range(NRB):
        srb[r_, rb, rb * 4 + r_ // D] = 1.0
    blob = np.zeros((128, 1440), np.float32)
    blob[:, 0:128] = np.eye(128)
    blob[:, 128:192] = srb.reshape(128, -1)
    blob[:FK, 192:704] = sel.reshape(FK, -1)
    blob[0, 704:832] = 1.0
    blob[0, 832:1432] = np.stack([b0, b1, b2]).reshape(-1)
    return w0, w1, w2, blob.astype(BF16)


def _prep_x_shard(x, c):
    xs = x[c * BC:(c + 1) * BC]                           # (BC, 39, 32)
    xt = xs.transpose(1, 0, 2).reshape(F0, ROWS)          # (39, 8192)
    xtr = xt.reshape(F0, NRT, RT)
    # z0: symmetric outer product, upper-triangle channels (m <= n) only;
    # channel c -> (kb, p) with c = 128*kb + p
    mi, ni = np.triu_indices(F0)
    z0 = np.zeros((KB0 * P0, ROWS), np.float32)
    z0[:len(mi)] = xt[mi] * xt[ni]
    z0 = z0.reshape(KB0, P0, NRT, RT)
    z0 = np.ascontiguousarray(z0.transpose(2, 1, 0, 3)).astype(BF16)
    # gather source slabs: xsrc[rt, j, i-i0, :] = x[min(m_start + 5*(i-i0)... , 38)]
    xsrc = np.zeros((NRT, len(GATHERS), 8, RT), np.float32)
    for j, (q, p0, p1, i0, i1, m0) in enumerate(GATHERS):
        for k in range(i1 - i0):
            xsrc[:, j, k, :] = xtr[min(m0 + 5 * k, F0 - 1)]
    return {'z0': z0, 'xsrc': xsrc.astype(BF16)}


def _build(with_bias=True):
    key = f'nc_{with_bias}'
    if key in _cached:
        return _cached[key]
    nc = bacc.Bacc("TRN2", target_bir_lowering=False, debug=False,
                   enable_asserts=False, num_devices=NCORES)
    ins = {
        'z0': nc.dram_tensor("z0", (NRT, P0, KB0, RT), mybir.dt.bfloat16,
                             kind="ExternalInput").ap(),
        'xsrc': nc.dram_tensor("xsrc", (NRT, len(GATHERS), 8, RT),
                               mybir.dt.bfloat16, kind="ExternalInput").ap(),
        'w0': nc.dram_tensor("w0", (P0, KB0, O), mybir.dt.bfloat16,
                             kind="ExternalInput").ap(),
        'w1': nc.dram_tensor("w1", (BL, NBL, O), mybir.dt.bfloat16,
                             kind="ExternalInput").ap(),
        'w2': nc.dram_tensor("w2", (BL, NBL, O), mybir.dt.bfloat16,
                             kind="ExternalInput").ap(),
        'blob': nc.dram_tensor("blob", (128, 1440), mybir.dt.bfloat16,
                               kind="ExternalInput").ap(),
    }
    outs = {
        'out': nc.dram_tensor("out", (NRT, BPT, 2 * O), mybir.dt.float32,
                              kind="ExternalOutput").ap(),
    }
    with tile.TileContext(nc, trace_sim=False) as tc:
        _emit(tc, outs, ins, with_bias=with_bias)
    nc.compile()
    _cached[key] = nc
    return nc


def kernel(x, W0, W1, W2, b0, b1, b2):
    bias_zero = not (np.any(b0) or np.any(b1) or np.any(b2))
    nc = _build(with_bias=not bias_zero)
    w0, w1, w2, blob = _prep_weights(
        np.asarray(W0, np.float32), np.asarray(W1, np.float32),
        np.asarray(W2, np.float32), np.asarray(b0, np.float32),
        np.asarray(b1, np.float32), np.asarray(b2, np.float32))
    x = np.asarray(x, np.float32)
    in_maps = []
    for c in range(NCORES):
        in_maps.append({
            **_prep_x_shard(x, c),
            'w0': w0, 'w1': w1, 'w2': w2, 'blob': blob,
        })
    res = bass_utils.run_bass_kernel_spmd(
        nc, in_maps, core_ids=list(range(NCORES)))
    out = np.empty((B, 2 * O), np.float32)
    for c in range(NCORES):
        out[c * BC:(c + 1) * BC, :] = res.results[c]['out'].reshape(BC, 2 * O)
    return out
